# revision 33
# baseline (speedup 1.0000x reference)
"""External Attention (nn_External_Attention) on 8 TRN2 NeuronCores.

kernel(x, Wk, Wv) -> x + Wv @ l1norm_M(softmax_N(Wk @ x))
  x  [16, 512, 4096] f32,  Wk [256, 512] f32,  Wv [512, 256] f32

Sharding: data-parallel over batch B=16 -> 2 batches per core across 8 cores.

Per-core pipeline v2 (C=512, M=256, N=4096), per batch b:
  A(b):  per 512-col tile j: logits = WkT.T @ x  (fp32r matmul, full PE rate),
         E = exp(logits - 1.5) on ACT written as fp8e4 interleaved [128,2,N]
         (the -1.5 bias cancels in the normalizations; keeps fp8 in range),
         with per-row partial sums accumulated to RSP.
  stats: rowsum -> rr = 1/rowsum (DVE), rr8 = rr*2^13 (fp8 interleaved),
         Wv8 = WvT * rr * 2^15 (fp8 interleaved).
  B(b):  per j: cs = rr8 . E   (ONE fp8 DoubleRow matmul, 0.5 cyc/row)
         rcs = reciprocal_approx_fast(cs) on DVE (no ACT table thrash)
         bc = (2^9 * ones) outer rcs  (PE broadcast to [128,512] PSUM)
         E'' = E * bc -> fp8 (DVE tensor_tensor, this IS the fp8 cast)
         po = Wv8 @DR E''  (fp8 DoubleRow)
         y = po * 2^-11 + x  (fused scalar_tensor_tensor on DVE/Pool)
  Emission interleaves B(b0) with A(b1) so the in-order PE queue never
  stalls on the per-j normalization chain.

Scales: E = e^-1.5 exp(l); rr8 = 2^13/rowsum; cs'' = 2^13 cs_eff;
  bc = 2^9/cs'' = 2^-4/cs_eff; E'' = E*bc; Wv8 = 2^15 Wv rr;
  O = Wv8 @ E'' = 2^11 (x-residual-free attention out); y = x + O*2^-11.
Attention term is only ~0.6% of ||y||, so fp8 errors are damped ~170x;
expected end-to-end rel err ~1e-4 level.
"""
from contextlib import ExitStack

import numpy as np

import concourse.bacc as bacc
import concourse.mybir as mybir
import concourse.tile as tile
from concourse.bass_utils import run_bass_kernel_spmd

F32 = mybir.dt.float32
F32R = mybir.dt.float32r
BF16 = mybir.dt.bfloat16
FP8 = mybir.dt.float8e4
AF = mybir.ActivationFunctionType
ALU = mybir.AluOpType
AX = mybir.AxisListType
DR = mybir.MatmulPerfMode.DoubleRow

B, C, M, N = 16, 512, 256, 4096
NCORES = 8
BPC = B // NCORES
KC = C // 128   # 4
KM = M // 128   # 2
NT = 512
NJ = N // NT    # 8
XW = 1024       # x dma tile width
NH = N // XW    # 4

EXP_BIAS = -4.0
S_RR = 6.0
S_WV = 512.0
DRAIN = S_RR / S_WV
IDENT_VAL = S_WV / S_RR


def _build(nc, y_add_plan=("v", "a", "v", "a", "v", "a", "v", "a")):
    """y_add_plan: per (j%2)*4+co slot: 'v'=DVE fused STT add, 'a'=PE identity
    residual + ACT copy drain."""
    x_d = nc.dram_tensor("x", [BPC, C, N], F32R, kind="ExternalInput").ap()
    wkT_d = nc.dram_tensor("wkT", [C, M], F32R, kind="ExternalInput").ap()
    wvT_d = nc.dram_tensor("wvT", [M, C], F32, kind="ExternalInput").ap()
    y_d = nc.dram_tensor("y", [BPC, C, N], F32, kind="ExternalOutput").ap()
    ident_d = nc.dram_tensor("ident", [128, 128], F32R, kind="ExternalInput").ap()

    with tile.TileContext(nc) as tc, ExitStack() as ctx:
        wpool = ctx.enter_context(tc.tile_pool(name="w", bufs=1))
        xpool = ctx.enter_context(tc.tile_pool(name="xp", bufs=BPC * NH * KC))
        epool = ctx.enter_context(tc.tile_pool(name="ep", bufs=2))
        eppool = ctx.enter_context(tc.tile_pool(name="epp", bufs=1))
        spool = ctx.enter_context(tc.tile_pool(name="sp", bufs=2))
        rcspool = ctx.enter_context(tc.tile_pool(name="rcs", bufs=4))
        bcpool = ctx.enter_context(tc.tile_pool(name="bcp", bufs=3))
        ypool = ctx.enter_context(tc.tile_pool(name="yp", bufs=5))
        ps_flex = ctx.enter_context(tc.tile_pool(name="ps_fx", bufs=5, space="PSUM"))
        ps_o = ctx.enter_context(tc.tile_pool(name="ps_o", bufs=3, space="PSUM"))

        wk_sb = []
        for kc in range(KC):
            t = wpool.tile([128, M], F32R, tag=f"wk{kc}", name=f"wk{kc}")
            nc.sync.dma_start(t[:], wkT_d[kc * 128:(kc + 1) * 128, :])
            wk_sb.append(t)
        ebias_sb = wpool.tile([128, 1], F32, tag="ebias", name="ebias")
        nc.gpsimd.memset(ebias_sb[:], EXP_BIAS)
        wv_sb = [wpool.tile([128, C], F32, tag=f"wv{km}", name=f"wv{km}")
                 for km in range(KM)]
        ident_sb = wpool.tile([128, 128], F32R, tag="ident", name="ident")

        def load_late_weights():
            for km in range(KM):
                nc.sync.dma_start(wv_sb[km][:], wvT_d[km * 128:(km + 1) * 128, :])
            nc.sync.dma_start(ident_sb[:], ident_d[:, :])

        X, E, EPP, RSP, RR8, WV8 = {}, {}, {}, {}, {}, {}
        RCS, YS = {}, {}

        def load_x(b, h, split=False):
            for kc in range(KC):
                if split:
                    # two [128, NT] loads so the very first matmuls start sooner
                    for jj in range(XW // NT):
                        t = xpool.tile([128, NT], F32R, tag=f"xs{jj}", bufs=KC,
                                       name=f"x{b}_{h}_{kc}_{jj}")
                        nc.sync.dma_start(
                            t[:], x_d[b, kc * 128:(kc + 1) * 128,
                                      h * XW + jj * NT:h * XW + (jj + 1) * NT])
                        X[(b, h, kc, jj)] = t
                else:
                    t = xpool.tile([128, XW], F32R, tag="x", bufs=(2 * NH - 1) * KC,
                                   name=f"x{b}_{h}_{kc}")
                    nc.sync.dma_start(
                        t[:], x_d[b, kc * 128:(kc + 1) * 128, h * XW:(h + 1) * XW])
                    X[(b, h, kc)] = t

        def xs(b, kc, j):
            h, jj = j // (XW // NT), j % (XW // NT)
            if (b, h, kc, jj) in X:
                return X[(b, h, kc, jj)][:, :]
            return X[(b, h, kc)][:, jj * NT:(jj + 1) * NT]

        def init_b(b):
            E[b] = epool.tile([128, KM, N], FP8, tag="e", name=f"e{b}")
            RSP[b] = [spool.tile([128, NJ], F32, tag=f"rsp{km}", name=f"rsp{b}_{km}")
                      for km in range(KM)]

        def emit_A(b, j):
            for km in range(KM):
                pl = ps_flex.tile([128, NT], F32, tag="fx", name=f"pl{b}_{j}_{km}")
                for kc in range(KC):
                    nc.tensor.matmul(pl[:], wk_sb[kc][:, km * 128:(km + 1) * 128],
                                     xs(b, kc, j),
                                     start=(kc == 0), stop=(kc == KC - 1))
                nc.scalar.activation(E[b][:, km, j * NT:(j + 1) * NT], pl[:],
                                     AF.Exp, bias=ebias_sb[:],
                                     accum_out=RSP[b][km][:, j:j + 1])

        def emit_stats(b):
            RR8[b] = spool.tile([128, KM, 1], FP8, tag="rr8", name=f"rr8{b}")
            WV8[b] = spool.tile([128, KM, C], FP8, tag="wv8", name=f"wv8{b}")
            for km in range(KM):
                rs = spool.tile([128, 1], F32, tag=f"rs{km}", name=f"rs{b}_{km}")
                nc.vector.tensor_reduce(rs[:], RSP[b][km][:], axis=AX.X, op=ALU.add)
                rr = spool.tile([128, 1], F32, tag=f"rr{km}", name=f"rr{b}_{km}")
                nc.vector.reciprocal(rr[:], rs[:])
                nc.vector.tensor_scalar_mul(RR8[b][:, km, :], rr[:], S_RR)
                rr15 = spool.tile([128, 1], F32, tag=f"rr15{km}", name=f"rr15{b}_{km}")
                nc.vector.tensor_scalar_mul(rr15[:], rr[:], S_WV)
                nc.vector.tensor_scalar_mul(WV8[b][:, km, :], wv_sb[km][:], rr15[:])

        def emit_cs(b, j):
            cs = ps_flex.tile([128, NT], F32, tag="fx", name=f"cs{b}_{j}")
            for km in range(KM):
                nc.tensor.matmul(cs[0:1, :], RR8[b][:, km, :],
                                 E[b][:, km, j * NT:(j + 1) * NT],
                                 start=(km == 0), stop=(km == KM - 1))
            rcs = rcspool.tile([1, NT], F32, tag="rcs", name=f"rcs{b}_{j}")
            nc.vector.reciprocal_approx_fast(out=rcs[:], in_=cs[0:1, :])
            RCS[(b, j)] = rcs

        def emit_bc(b, j):
            if b not in EPP:
                EPP[b] = eppool.tile([128, KM, N], FP8, tag="epp", name=f"epp{b}")
            bc = bcpool.tile([128, NT], F32, tag="bc", name=f"bc{b}_{j}")
            rcs = RCS.pop((b, j))
            nc.gpsimd.partition_broadcast(bc[:], rcs[:])
            bc3 = bc[:, None, :].broadcast_to((128, KM, NT))
            nc.vector.tensor_tensor(EPP[b][:, :, j * NT:(j + 1) * NT],
                                    E[b][:, :, j * NT:(j + 1) * NT],
                                    bc3, op=ALU.mult)

        def emit_mm2(b, j, fine=False):
            if fine:
                ys_t = [ypool.tile([128, NT], F32, tag="yf", bufs=4,
                                   name=f"yf{b}_{j}_{co}")
                        for co in range(KC)]
            elif j % 2 == 0:
                YS[b] = [ypool.tile([128, XW], F32, tag="y",
                                    name=f"y{b}_{j}_{co}")
                         for co in range(KC)]
            for co in range(KC):
                po = ps_o.tile([128, NT], F32, tag="po", name=f"po{b}_{j}_{co}")
                on_act = y_add_plan[(j % 2) * 4 + co] == "a"
                nc.tensor.matmul(po[:], WV8[b][:, :, co * 128:(co + 1) * 128],
                                 EPP[b][:, :, j * NT:(j + 1) * NT],
                                 start=True, stop=not on_act, perf_mode=DR)
                if fine:
                    ys_slice = ys_t[co][:, :]
                else:
                    ys_slice = YS[b][co][:, (j % 2) * NT:(j % 2 + 1) * NT]
                if on_act:
                    # residual rides the PE: po += (S_WV/S_RR)*x, then copy-out
                    nc.tensor.matmul(po[:], ident_sb[:], xs(b, co, j),
                                     start=False, stop=True)
                    nc.scalar.activation(ys_slice, po[:], AF.Copy,
                                         scale=DRAIN)
                else:
                    nc.vector.scalar_tensor_tensor(
                        ys_slice, po[:], DRAIN, xs(b, co, j).bitcast(F32),
                        op0=ALU.mult, op1=ALU.add)
            if fine:
                for co in range(KC):
                    nc.sync.dma_start(
                        y_d[b, co * 128:(co + 1) * 128, j * NT:(j + 1) * NT],
                        ys_t[co][:])
            elif j % 2 == 1:
                h = j // 2
                for co in range(KC):
                    nc.sync.dma_start(
                        y_d[b, co * 128:(co + 1) * 128, h * XW:(h + 1) * XW],
                        YS[b][co][:])

        # ---- phase 1: A(b0), with all x prefetched (stores can't block loads) ----
        load_x(0, 0, split=True)
        for h in range(1, NH):
            load_x(0, h)
        load_late_weights()
        for h in range(NH):
            load_x(1, h)
        init_b(0)
        for j in range(NJ):
            emit_A(0, j)
        emit_stats(0)

        # ---- phase 2: B(b0) interleaved with A(b1) ----
        init_b(1)
        emit_cs(0, 0)
        for j in range(NJ):
            emit_A(1, j)
            if j + 1 < NJ:
                emit_cs(0, j + 1)
            emit_bc(0, j)
            emit_mm2(0, j)
        emit_stats(1)

        # ---- phase 3: B(b1) ----
        emit_cs(1, 0)
        for j in range(NJ):
            if j + 1 < NJ:
                emit_cs(1, j + 1)
            emit_bc(1, j)
            emit_mm2(1, j, fine=(j >= NJ - 2))
    return nc


_CACHE = {}


def _get_program():
    if "nc" not in _CACHE:
        nc = bacc.Bacc("TRN2", target_bir_lowering=False, debug=False,
                       enable_asserts=True)
        _build(nc)
        nc.compile()
        _CACHE["nc"] = nc
    return _CACHE["nc"]


def kernel(x, Wk, Wv):
    x = np.ascontiguousarray(np.asarray(x), dtype=np.float32)
    wkT = np.ascontiguousarray(np.asarray(Wk, dtype=np.float32).T)
    wvT = np.ascontiguousarray(np.asarray(Wv, dtype=np.float32).T)

    nc = _get_program()
    ident = np.ascontiguousarray((512.0 / 6.0) * np.eye(128, dtype=np.float32))
    in_maps = [{"x": x[i * BPC:(i + 1) * BPC], "wkT": wkT, "wvT": wvT,
                "ident": ident}
               for i in range(NCORES)]
    res = run_bass_kernel_spmd(nc, in_maps, list(range(NCORES)))
    y = np.concatenate([res.results[i]["y"] for i in range(NCORES)], axis=0)
    return np.ascontiguousarray(y, dtype=np.float32)


# revision 34
# speedup vs baseline: 1.0901x; 1.0901x over previous
"""External Attention (nn_External_Attention) on 8 TRN2 NeuronCores.

kernel(x, Wk, Wv) -> x + Wv @ l1norm_M(softmax_N(Wk @ x))
  x  [16, 512, 4096] f32,  Wk [256, 512] f32,  Wv [512, 256] f32

Sharding: data-parallel over batch B=16 -> 2 batches per core across 8 cores.

Per-core pipeline v2 (C=512, M=256, N=4096), per batch b:
  A(b):  per 512-col tile j: logits = WkT.T @ x  (fp32r matmul, full PE rate),
         E = exp(logits - 1.5) on ACT written as fp8e4 interleaved [128,2,N]
         (the -1.5 bias cancels in the normalizations; keeps fp8 in range),
         with per-row partial sums accumulated to RSP.
  stats: rowsum -> rr = 1/rowsum (DVE), rr8 = rr*2^13 (fp8 interleaved),
         Wv8 = WvT * rr * 2^15 (fp8 interleaved).
  B(b):  per j: cs = rr8 . E   (ONE fp8 DoubleRow matmul, 0.5 cyc/row)
         rcs = reciprocal_approx_fast(cs) on DVE (no ACT table thrash)
         bc = (2^9 * ones) outer rcs  (PE broadcast to [128,512] PSUM)
         E'' = E * bc -> fp8 (DVE tensor_tensor, this IS the fp8 cast)
         po = Wv8 @DR E''  (fp8 DoubleRow)
         y = po * 2^-11 + x  (fused scalar_tensor_tensor on DVE/Pool)
  Emission interleaves B(b0) with A(b1) so the in-order PE queue never
  stalls on the per-j normalization chain.

Scales: E = e^-1.5 exp(l); rr8 = 2^13/rowsum; cs'' = 2^13 cs_eff;
  bc = 2^9/cs'' = 2^-4/cs_eff; E'' = E*bc; Wv8 = 2^15 Wv rr;
  O = Wv8 @ E'' = 2^11 (x-residual-free attention out); y = x + O*2^-11.
Attention term is only ~0.6% of ||y||, so fp8 errors are damped ~170x;
expected end-to-end rel err ~1e-4 level.
"""
from contextlib import ExitStack

import numpy as np

import concourse.bacc as bacc
import concourse.mybir as mybir
import concourse.tile as tile
from concourse.bass_utils import run_bass_kernel_spmd

F32 = mybir.dt.float32
F32R = mybir.dt.float32r
BF16 = mybir.dt.bfloat16
FP8 = mybir.dt.float8e4
AF = mybir.ActivationFunctionType
ALU = mybir.AluOpType
AX = mybir.AxisListType
DR = mybir.MatmulPerfMode.DoubleRow

B, C, M, N = 16, 512, 256, 4096
NCORES = 8
BPC = B // NCORES
KC = C // 128   # 4
KM = M // 128   # 2
NT = 512
NJ = N // NT    # 8
XW = 1024       # x dma tile width
NH = N // XW    # 4

EXP_BIAS = -4.0
S_RR = 6.0
S_WV = 512.0
DRAIN = S_RR / S_WV
IDENT_VAL = S_WV / S_RR


def _build(nc, y_add_plan=("v", "a", "v", "a", "v", "a", "v", "a")):
    """y_add_plan: per (j%2)*4+co slot: 'v'=DVE fused STT add, 'a'=PE identity
    residual + ACT copy drain."""
    x_d = nc.dram_tensor("x", [BPC, C, N], F32R, kind="ExternalInput").ap()
    wkT_d = nc.dram_tensor("wkT", [C, M], F32R, kind="ExternalInput").ap()
    wvT_d = nc.dram_tensor("wvT", [M, C], F32, kind="ExternalInput").ap()
    y_d = nc.dram_tensor("y", [BPC, C, N], F32, kind="ExternalOutput").ap()
    ident_d = nc.dram_tensor("ident", [128, 128], F32R, kind="ExternalInput").ap()

    with tile.TileContext(nc) as tc, ExitStack() as ctx:
        wpool = ctx.enter_context(tc.tile_pool(name="w", bufs=1))
        xpool = ctx.enter_context(tc.tile_pool(name="xp", bufs=BPC * NH * KC))
        epool = ctx.enter_context(tc.tile_pool(name="ep", bufs=2))
        eppool = ctx.enter_context(tc.tile_pool(name="epp", bufs=1))
        spool = ctx.enter_context(tc.tile_pool(name="sp", bufs=2))
        rcspool = ctx.enter_context(tc.tile_pool(name="rcs", bufs=4))
        bcpool = ctx.enter_context(tc.tile_pool(name="bcp", bufs=3))
        ypool = ctx.enter_context(tc.tile_pool(name="yp", bufs=5))
        ps_flex = ctx.enter_context(tc.tile_pool(name="ps_fx", bufs=4, space="PSUM"))
        ps_o = ctx.enter_context(tc.tile_pool(name="ps_o", bufs=4, space="PSUM"))

        wk_sb = []
        for kc in range(KC):
            t = wpool.tile([128, M], F32R, tag=f"wk{kc}", name=f"wk{kc}")
            nc.sync.dma_start(t[:], wkT_d[kc * 128:(kc + 1) * 128, :])
            wk_sb.append(t)
        ebias_sb = wpool.tile([128, 1], F32, tag="ebias", name="ebias")
        nc.gpsimd.memset(ebias_sb[:], EXP_BIAS)
        wv_sb = [wpool.tile([128, C], F32, tag=f"wv{km}", name=f"wv{km}")
                 for km in range(KM)]
        ident_sb = wpool.tile([128, 128], F32R, tag="ident", name="ident")

        def load_late_weights():
            for km in range(KM):
                nc.sync.dma_start(wv_sb[km][:], wvT_d[km * 128:(km + 1) * 128, :])
            nc.sync.dma_start(ident_sb[:], ident_d[:, :])

        X, E, EPP, RSP, RR8, WV8 = {}, {}, {}, {}, {}, {}
        RCS, YS = {}, {}

        def load_x(b, h, split=False):
            for kc in range(KC):
                if split:
                    # two [128, NT] loads so the very first matmuls start sooner
                    for jj in range(XW // NT):
                        t = xpool.tile([128, NT], F32R, tag=f"xs{jj}", bufs=KC,
                                       name=f"x{b}_{h}_{kc}_{jj}")
                        nc.sync.dma_start(
                            t[:], x_d[b, kc * 128:(kc + 1) * 128,
                                      h * XW + jj * NT:h * XW + (jj + 1) * NT])
                        X[(b, h, kc, jj)] = t
                else:
                    t = xpool.tile([128, XW], F32R, tag="x", bufs=(2 * NH - 1) * KC,
                                   name=f"x{b}_{h}_{kc}")
                    nc.sync.dma_start(
                        t[:], x_d[b, kc * 128:(kc + 1) * 128, h * XW:(h + 1) * XW])
                    X[(b, h, kc)] = t

        def xs(b, kc, j):
            h, jj = j // (XW // NT), j % (XW // NT)
            if (b, h, kc, jj) in X:
                return X[(b, h, kc, jj)][:, :]
            return X[(b, h, kc)][:, jj * NT:(jj + 1) * NT]

        def init_b(b):
            E[b] = epool.tile([128, KM, N], FP8, tag="e", name=f"e{b}")
            RSP[b] = [spool.tile([128, NJ], F32, tag=f"rsp{km}", name=f"rsp{b}_{km}")
                      for km in range(KM)]

        def emit_A(b, j):
            for km in range(KM):
                pl = ps_flex.tile([128, NT], F32, tag="fx", name=f"pl{b}_{j}_{km}")
                for kc in range(KC):
                    nc.tensor.matmul(pl[:], wk_sb[kc][:, km * 128:(km + 1) * 128],
                                     xs(b, kc, j),
                                     start=(kc == 0), stop=(kc == KC - 1))
                nc.scalar.activation(E[b][:, km, j * NT:(j + 1) * NT], pl[:],
                                     AF.Exp, bias=ebias_sb[:],
                                     accum_out=RSP[b][km][:, j:j + 1])

        def emit_stats(b):
            RR8[b] = spool.tile([128, KM, 1], FP8, tag="rr8", name=f"rr8{b}")
            WV8[b] = spool.tile([128, KM, C], FP8, tag="wv8", name=f"wv8{b}")
            for km in range(KM):
                rs = spool.tile([128, 1], F32, tag=f"rs{km}", name=f"rs{b}_{km}")
                nc.vector.tensor_reduce(rs[:], RSP[b][km][:], axis=AX.X, op=ALU.add)
                rr = spool.tile([128, 1], F32, tag=f"rr{km}", name=f"rr{b}_{km}")
                nc.vector.reciprocal(rr[:], rs[:])
                nc.vector.tensor_scalar_mul(RR8[b][:, km, :], rr[:], S_RR)
                rr15 = spool.tile([128, 1], F32, tag=f"rr15{km}", name=f"rr15{b}_{km}")
                nc.vector.tensor_scalar_mul(rr15[:], rr[:], S_WV)
                nc.vector.tensor_scalar_mul(WV8[b][:, km, :], wv_sb[km][:], rr15[:])

        def emit_cs(b, j):
            cs = ps_flex.tile([128, NT], F32, tag="fx", name=f"cs{b}_{j}")
            for km in range(KM):
                nc.tensor.matmul(cs[0:1, :], RR8[b][:, km, :],
                                 E[b][:, km, j * NT:(j + 1) * NT],
                                 start=(km == 0), stop=(km == KM - 1))
            rcs = rcspool.tile([1, NT], F32, tag="rcs", name=f"rcs{b}_{j}")
            nc.vector.reciprocal_approx_fast(out=rcs[:], in_=cs[0:1, :])
            RCS[(b, j)] = rcs

        def emit_bc(b, j):
            if b not in EPP:
                EPP[b] = eppool.tile([128, KM, N], FP8, tag="epp", name=f"epp{b}")
            bc = bcpool.tile([128, NT], F32, tag="bc", name=f"bc{b}_{j}")
            rcs = RCS.pop((b, j))
            nc.gpsimd.partition_broadcast(bc[:], rcs[:])
            bc3 = bc[:, None, :].broadcast_to((128, KM, NT))
            nc.vector.tensor_tensor(EPP[b][:, :, j * NT:(j + 1) * NT],
                                    E[b][:, :, j * NT:(j + 1) * NT],
                                    bc3, op=ALU.mult)

        def emit_mm2(b, j, fine=False):
            if fine:
                ys_t = [ypool.tile([128, NT], F32, tag="yf", bufs=4,
                                   name=f"yf{b}_{j}_{co}")
                        for co in range(KC)]
            elif j % 2 == 0:
                YS[b] = [ypool.tile([128, XW], F32, tag="y",
                                    name=f"y{b}_{j}_{co}")
                         for co in range(KC)]
            for co in range(KC):
                po = ps_o.tile([128, NT], F32, tag="po", name=f"po{b}_{j}_{co}")
                on_act = y_add_plan[(j % 2) * 4 + co] == "a"
                nc.tensor.matmul(po[:], WV8[b][:, :, co * 128:(co + 1) * 128],
                                 EPP[b][:, :, j * NT:(j + 1) * NT],
                                 start=True, stop=not on_act, perf_mode=DR)
                if fine:
                    ys_slice = ys_t[co][:, :]
                else:
                    ys_slice = YS[b][co][:, (j % 2) * NT:(j % 2 + 1) * NT]
                if on_act:
                    # residual rides the PE: po += (S_WV/S_RR)*x, then copy-out
                    nc.tensor.matmul(po[:], ident_sb[:], xs(b, co, j),
                                     start=False, stop=True)
                    nc.scalar.activation(ys_slice, po[:], AF.Copy,
                                         scale=DRAIN)
                else:
                    nc.vector.scalar_tensor_tensor(
                        ys_slice, po[:], DRAIN, xs(b, co, j).bitcast(F32),
                        op0=ALU.mult, op1=ALU.add)
            if fine:
                for co in range(KC):
                    nc.sync.dma_start(
                        y_d[b, co * 128:(co + 1) * 128, j * NT:(j + 1) * NT],
                        ys_t[co][:])
            elif j % 2 == 1:
                h = j // 2
                for co in range(KC):
                    nc.sync.dma_start(
                        y_d[b, co * 128:(co + 1) * 128, h * XW:(h + 1) * XW],
                        YS[b][co][:])

        # ---- phase 1: A(b0), with all x prefetched (stores can't block loads) ----
        load_x(0, 0, split=True)
        for h in range(1, NH):
            load_x(0, h)
        load_late_weights()
        for h in range(NH):
            load_x(1, h)
        init_b(0)
        for j in range(NJ):
            emit_A(0, j)
        emit_stats(0)

        # ---- phase 2: B(b0) interleaved with A(b1) ----
        init_b(1)
        emit_cs(0, 0)
        for j in range(NJ):
            emit_A(1, j)
            if j + 1 < NJ:
                emit_cs(0, j + 1)
            emit_bc(0, j)
            emit_mm2(0, j)
        emit_stats(1)

        # ---- phase 3: B(b1) ----
        emit_cs(1, 0)
        for j in range(NJ):
            if j + 1 < NJ:
                emit_cs(1, j + 1)
            emit_bc(1, j)
            emit_mm2(1, j, fine=(j >= NJ - 2))
    return nc


_CACHE = {}


def _get_program():
    if "nc" not in _CACHE:
        nc = bacc.Bacc("TRN2", target_bir_lowering=False, debug=False,
                       enable_asserts=True)
        _build(nc)
        nc.compile()
        _CACHE["nc"] = nc
    return _CACHE["nc"]


def kernel(x, Wk, Wv):
    x = np.ascontiguousarray(np.asarray(x), dtype=np.float32)
    wkT = np.ascontiguousarray(np.asarray(Wk, dtype=np.float32).T)
    wvT = np.ascontiguousarray(np.asarray(Wv, dtype=np.float32).T)

    nc = _get_program()
    ident = np.ascontiguousarray((512.0 / 6.0) * np.eye(128, dtype=np.float32))
    in_maps = [{"x": x[i * BPC:(i + 1) * BPC], "wkT": wkT, "wvT": wvT,
                "ident": ident}
               for i in range(NCORES)]
    res = run_bass_kernel_spmd(nc, in_maps, list(range(NCORES)))
    y = np.concatenate([res.results[i]["y"] for i in range(NCORES)], axis=0)
    return np.ascontiguousarray(y, dtype=np.float32)
